# revision 4
# baseline (speedup 1.0000x reference)
"""Trainium2 Bass kernel v2 for nn_Attention_39865886442202 (periodic local attention).

Design (per core, one batch element):
  - Mask folded into the QK matmul: K-ext tiles carry 8 onehot rows, Q-ext tiles
    carry 8 mask-value rows (3 variants for chunk offsets ci in {-1,0,+1}),
    giving a single 40-row-contraction matmul per (tile, chunk, head).
  - 2 heads per 128-partition "pair" tile at slots {rows 0-40, rows 64-104};
    QKV projection uses zero-padded weight columns so PSUM rows land directly
    on the slots (one evacuation copy per (nf, pair-block)).
  - Query-column windows per chunk: center full 128; e- cols [0, 16*jhi);
    e+ cols [16*jlo2, 128) with jlo2 even (AV partition-alignment rules).
  - Scores for 4 heads packed into one 3-bank PSUM tile; ONE exp per group.
  - AV with interleaved ones-column in V gives softmax denominators.
  - Per-tile output chain: transpose -> project -> bias -> scattered DMA store.
"""

import numpy as np
import ml_dtypes

import concourse.bass as bass
import concourse.mybir as mybir
import concourse.tile as tile
from concourse import bacc, bass_utils

DIM = 256
NUM_HEADS = 8
HEAD_DIM = 32
SCALE = HEAD_DIM ** (-0.5)
W_BUF = 160
WK = 11
B = 8
N = 2048
W = 128
T = N // W         # 16 token blocks
NTILES = 16        # query tiles (8 residues x 16 t)
NEG = -30000.0

bf = ml_dtypes.bfloat16
f32 = mybir.dt.float32
bf16 = mybir.dt.bfloat16

_CACHE = {}
LAST_EXEC_NS = None
LAST_RES = None


def _build_m1(w):
    Wb = W_BUF
    mask = np.ones((Wb, Wb), dtype=np.float32)
    for i in range(Wb):
        b = i - WK // 2 if i - WK // 2 > 0 else 0
        if b > Wb - WK:
            b = Wb - WK
        mask[i, b:b + WK] = 0.0
    mask[mask >= 1] = -np.inf
    off = (Wb - w) // 2
    m1 = mask[off:Wb - off, off:Wb - off].copy()
    e = WK // 2 + 1
    m1[:e] = mask[:e, :w]
    m1[-e:] = mask[-e:, -w:]
    return m1  # [128, 128], 0 = visible, -inf = masked


def _windows():
    """Per (tile k): ordered blocks [(cg, jlo2, jhi)] — center first."""
    m1 = _build_m1(W)
    wins = {}
    for k in range(NTILES):
        blocks = []
        for cg in (k, k - 1, k + 1):
            if not (0 <= cg < NTILES):
                continue
            js = [j for j in range(8)
                  if (m1[8 * k + j, 8 * cg:8 * cg + 8] == 0).any()]
            assert js == list(range(min(js), max(js) + 1))
            jlo, jhi = min(js), max(js) + 1
            if cg == k:
                assert (jlo, jhi) == (0, 8)
            if cg > k:
                assert jhi == 8
                jlo = jlo - (jlo % 2)
            if cg < k:
                assert jlo == 0
            blocks.append((cg, jlo, jhi))
        wins[k] = blocks
    return wins


def _host_consts():
    m1 = _build_m1(W)
    m1neg = np.where(m1 == 0.0, 0.0, NEG).astype(np.float32)
    # mask rows: maskr[v][r, p*16 + t] = m1[p, 8*(p//8 + v - 1) + r]
    maskr = np.full((3, 8, N), NEG, dtype=np.float32)
    for v in range(3):
        ci = v - 1
        for p in range(W):
            cg = p // 8 + ci
            if not (0 <= cg < NTILES):
                continue
            for r in range(8):
                maskr[v, r, p * 16:(p + 1) * 16] = m1neg[p, 8 * cg + r]
    # onehot rows: oh[r, c*16 + s] = 1 iff c % 8 == r
    oh = np.zeros((8, N), dtype=np.float32)
    for c in range(W):
        oh[c % 8, c * 16:(c + 1) * 16] = 1.0
    ident = np.eye(128, dtype=np.float32)
    return (maskr.reshape(24, N).astype(bf), oh.astype(bf), ident.astype(bf))


def _reorder_weights(Wqkv):
    """[256, 1280]: 4 padded q-pair blocks, 4 padded k-pair blocks, V."""
    Wq = Wqkv[:, :DIM] * SCALE
    Wk = Wqkv[:, DIM:2 * DIM]
    Wv = Wqkv[:, 2 * DIM:]
    out = np.zeros((DIM, 1280), dtype=np.float32)
    for g in range(4):
        out[:, 128 * g + 0:128 * g + 32] = Wq[:, 32 * (2 * g):32 * (2 * g + 1)]
        out[:, 128 * g + 64:128 * g + 96] = Wq[:, 32 * (2 * g + 1):32 * (2 * g + 2)]
        out[:, 512 + 128 * g + 0:512 + 128 * g + 32] = Wk[:, 32 * (2 * g):32 * (2 * g + 1)]
        out[:, 512 + 128 * g + 64:512 + 128 * g + 96] = Wk[:, 32 * (2 * g + 1):32 * (2 * g + 2)]
    out[:, 1024:1280] = Wv
    return out


def _pack_groups(blocks, heads):
    """Pack per-head block lists into one <=1024-col (2-bank) PSUM tile.
    HW constraint: matmuls sharing a PSUM bank must share a PE row-tile, so a
    bank may only hold blocks of one slot (head parity); bump to the next bank
    on slot change. Returns ([(h_idx, cg, jlo, jhi, off, width)], [runs])."""
    out = []
    runs = []
    off = 0
    bank_slot = {}
    for h_idx, h in enumerate(heads):
        sl = h % 2
        for (cg, jlo, jhi) in blocks:
            width = 16 * (jhi - jlo)
            if off // 512 != (off + width - 1) // 512:
                off = (off // 512 + 1) * 512  # bump past bank boundary
            if bank_slot.get(off // 512, sl) != sl:
                off = (off // 512 + 1) * 512  # bump on slot change
            bank_slot[off // 512] = sl
            out.append((h_idx, cg, jlo, jhi, off, width))
            if runs and runs[-1][1] == off:
                runs[-1] = (runs[-1][0], off + width)
            else:
                runs.append((off, off + width))
            off += width
    assert off <= 1024, off
    return out, runs


def _build_program(stage=4, ntiles=NTILES, noexp=False, nblocks=None):
    wins = _windows()
    nc = bacc.Bacc(None, target_bir_lowering=False)

    x_in = nc.declare_dram_parameter("x", [N, DIM], f32, isOutput=False)
    wqkvr_in = nc.declare_dram_parameter("wqkvr", [DIM, 1280], bf16, isOutput=False)
    wproj_in = nc.declare_dram_parameter("wproj", [DIM, DIM], bf16, isOutput=False)
    bproj_in = nc.declare_dram_parameter("bproj", [DIM], f32, isOutput=False)
    maskr_in = nc.declare_dram_parameter("maskr", [24, N], bf16, isOutput=False)
    oh_in = nc.declare_dram_parameter("oh", [8, N], bf16, isOutput=False)
    ident_in = nc.declare_dram_parameter("ident", [128, 128], bf16, isOutput=False)
    out_ext = nc.declare_dram_parameter("out", [N, DIM], bf16, isOutput=True)

    with tile.TileContext(nc) as tc:
        with (
            tc.tile_pool(name="sing", bufs=1) as sing,
            tc.tile_pool(name="sbp", bufs=2) as sbp,
            tc.tile_pool(name="small", bufs=4) as small,
            tc.tile_pool(name="psS", bufs=3, space="PSUM") as psS,
            tc.tile_pool(name="psU", bufs=2, space="PSUM") as psU,
        ):
            # ---- act table preload ----
            scratch = small.tile([128, 1], f32, tag="scr")
            nc.vector.memset(scratch, 0.0)
            warm = small.tile([128, 1], f32, tag="scr2")
            nc.scalar.activation(warm, scratch, mybir.ActivationFunctionType.Exp)

            # ---- x load first (the long pole), 4 chunks ----
            xbf = sing.tile([128, T * DIM], bf16)
            xin = x_in.rearrange("(t p) d -> p t d", p=128)
            xbf3 = xbf.rearrange("p (t d) -> p t d", t=T)
            for t4 in range(4):
                nc.gpsimd.dma_start(out=xbf3[:, 4 * t4:4 * (t4 + 1), :],
                                    in_=xin[:, 4 * t4:4 * (t4 + 1), :])
            ident_sb = sing.tile([128, 128], bf16)
            nc.sync.dma_start(out=ident_sb, in_=ident_in[:, :])
            wqkv_sb = []
            for dc in range(2):
                t_ = sing.tile([128, 1280], bf16, tag=f"wqkv{dc}")
                nc.sync.dma_start(out=t_, in_=wqkvr_in[128 * dc:128 * (dc + 1), :])
                wqkv_sb.append(t_)
            wproj_sb = []
            for dc in range(2):
                t_ = sing.tile([128, DIM], bf16, tag=f"wproj{dc}")
                nc.sync.dma_start(out=t_, in_=wproj_in[128 * dc:128 * (dc + 1), :])
                wproj_sb.append(t_)
            bias_rep = sing.tile([128, DIM], f32)
            bp = bproj_in[:]
            bproj_bcast = bass.AP(tensor=bp.tensor, offset=bp.offset,
                                  ap=[[0, 128], [1, DIM]])
            nc.gpsimd.dma_start(out=bias_rep, in_=bproj_bcast)
            # xTg: residue-major x^T (col = c*16 + s, token n = s*128 + c),
            # written directly by scattering the transpose evacuations.
            xTg = [sing.tile([128, N], bf16, name=f"xTg{dc}", tag=f"xTg{dc}")
                   for dc in range(2)]
            for t4 in range(4):
                for dc in range(2):
                    tp = psU.tile([128, 512], bf16, tag="u")
                    for i in range(4):
                        t = 4 * t4 + i
                        nc.tensor.transpose(
                            tp[:, 128 * i:128 * (i + 1)],
                            xbf[:, 256 * t + 128 * dc:256 * t + 128 * (dc + 1)],
                            ident_sb)
                    dt_ = xTg[dc][:, :]
                    dst = bass.AP(tensor=dt_.tensor, offset=dt_.offset + 4 * t4,
                                  ap=[list(dt_.ap[0]), [1, 4], [16, 128]])
                    sv = tp[:, :]
                    src = bass.AP(tensor=sv.tensor, offset=sv.offset,
                                  ap=[list(sv.ap[0]), [128, 4], [1, 128]])
                    nc.vector.tensor_copy(dst, src)

            # ---- QK projection into Qext/Kext pair tiles (per-pair chain:
            # staging copies -> v1 mask + onehot rows -> replication -> v0/v2
            # mask rows), so attention unblocks pair by pair ----
            qext = [sing.tile([128, 4 * N], bf16, name=f"qext{v}", tag=f"qext{v}") for v in range(3)]
            kext = sing.tile([128, 4 * N], bf16)

            def bcast_rows(dst_tile, r0, nrows, src_rows, c0, npair):
                dv = dst_tile[r0:r0 + nrows, 2048 * c0:2048 * (c0 + npair)]
                dst = bass.AP(tensor=dv.tensor, offset=dv.offset,
                              ap=[list(dv.ap[0]), [N, npair], [1, N]])
                sv = src_rows
                src = bass.AP(tensor=sv.tensor, offset=sv.offset,
                              ap=[list(sv.ap[0]), [0, npair], [1, N]])
                nc.sync.dma_start(out=dst, in_=src)

            # slot-1 mask/onehot rows preloaded (staging leaves rows 96+ alone)
            for v in range(3):
                bcast_rows(qext[v], 96, 8, maskr_in[8 * v:8 * v + 8, :], 0, 4)
            bcast_rows(kext, 96, 8, oh_in[:, :], 0, 4)

            for g in range(4):        # pair index
                for qk in range(2):   # 0 = q, 1 = k
                    blk = 4 * qk + g
                    for nf2 in range(2):
                        ps = psS.tile([128, 1024], f32, tag="sc",
                                      name=f"qkps{blk}{nf2}")
                        for half in range(2):
                            nf = 2 * nf2 + half
                            for dc in range(2):
                                nc.tensor.matmul(
                                    ps[:, 512 * half:512 * (half + 1)],
                                    lhsT=wqkv_sb[dc][:, 128 * blk:128 * (blk + 1)],
                                    rhs=xTg[dc][:, 512 * nf:512 * (nf + 1)],
                                    start=(dc == 0), stop=(dc == 1),
                                )
                        # contiguous evac (xTg order == qext/kext col order);
                        # rows [0:96] only so slot-1 mask/onehot rows survive
                        dtile = qext[1] if qk == 0 else kext
                        dst = dtile[0:96, 2048 * g + 1024 * nf2:
                                    2048 * g + 1024 * (nf2 + 1)]
                        if qk == 0:
                            nc.scalar.copy(dst, ps[0:96, :])
                        else:
                            nc.vector.tensor_copy(dst, ps[0:96, :])
                if g in (1, 3):
                    # per-half chain: slot-0 mask/onehot rows + replication
                    c0 = g - 1
                    for v in range(3):
                        bcast_rows(qext[v], 32, 8, maskr_in[8 * v:8 * v + 8, :], c0, 2)
                    bcast_rows(kext, 32, 8, oh_in[:, :], c0, 2)
                    for v in (0, 2):
                        for r0 in (0, 64):
                            nc.sync.dma_start(
                                out=qext[v][r0:r0 + 32, 2048 * c0:2048 * (c0 + 2)],
                                in_=qext[1][r0:r0 + 32, 2048 * c0:2048 * (c0 + 2)])
            # ---- V projection (keys c-major), interleaved ones column ----
            vsb = sing.tile([128, 16 * 264], bf16)
            vsb4 = vsb.rearrange("p (m h e) -> p m h e", m=16, e=33)
            nc.vector.memset(vsb4[:, :, :, 32:33], 1.0)

            def emit_v(m):
                ps = psS.tile([128, DIM], f32, tag="sc", name=f"vps{m}")
                for dc in range(2):
                    nc.tensor.matmul(
                        ps,
                        lhsT=xTg[dc][:, 128 * m:128 * (m + 1)],
                        rhs=wqkv_sb[dc][:, 1024:1280],
                        start=(dc == 0), stop=(dc == 1),
                    )
                nc.vector.tensor_copy(
                    vsb4[:, m, :, 0:32],
                    ps.rearrange("p (h e) -> p h e", h=NUM_HEADS),
                )

            for m in range(3):
                emit_v(m)

            # ---- attention (one-tile software pipeline) ----
            # heads grouped by slot: matmuls sharing a PSUM bank must
            # use the same PE row-tile (hw constraint)
            GROUPS = [(0, 2, 4), (1, 3, 5), (6, 7)]
            state = {}
            obufs = [sing.tile([128, 4 * DIM], bf16, name=f"obuf{i}",
                               tag=f"obuf{i}") for i in range(4)]

            def emit_scores_group(k, gi):
                heads = GROUPS[gi]
                pack, runs = _pack_groups(wins[k], heads)
                if nblocks is not None:
                    pack = pack[:nblocks]
                    runs = runs[:1]
                sps = psS.tile([128, 1024], f32, tag="sc", name=f"sps{k}g{gi}")
                for (hh, cg, jlo, jhi, off, width) in pack:
                    h = heads[hh]
                    g, sl = h // 2, h % 2
                    v = cg - k + 1
                    lhsT = kext[64 * sl:64 * sl + 40,
                                2048 * g + 128 * cg:2048 * g + 128 * (cg + 1)]
                    rhs = qext[v][64 * sl:64 * sl + 40,
                                  2048 * g + 128 * k + 16 * jlo:
                                  2048 * g + 128 * k + 16 * jhi]
                    nc.tensor.matmul(sps[:, off:off + width], lhsT=lhsT,
                                     rhs=rhs, start=True, stop=True)
                ptil = sbp.tile([128, 1024], bf16, tag="ptil", bufs=6, name=f"pt{k}g{gi}")
                for (a, b_) in runs:
                    if noexp:
                        nc.vector.tensor_copy(ptil[:, a:b_], sps[:, a:b_])
                    else:
                        nc.scalar.activation(ptil[:, a:b_], sps[:, a:b_],
                                             mybir.ActivationFunctionType.Exp)
                if k not in state:
                    state[k] = ([], [], None)
                state[k][0].append(ptil)
                state[k][1].append(pack)

            def emit_scores(k):
                for gi in range(len(GROUPS)):
                    emit_scores_group(k, gi)

            def emit_av_group(k, gi):
                ptils, packs, av = state[k][:3]
                if av is None:
                    av = psU.tile([128, 512], f32, tag="u", name=f"av{k}")
                    state[k] = (ptils, packs, av)
                heads = GROUPS[gi]
                if True:
                    for hh, h in enumerate(heads):
                        blocks = [b for b in packs[gi] if b[0] == hh]
                        mms = []
                        for (bh, cg, jlo, jhi, off, width) in blocks:
                            if jlo == 0:
                                mms.append(((0, 16 * jhi), off, None, cg))
                            else:
                                rl = 16 * jlo
                                if rl < 64:
                                    mms.append(((rl, 64), off, (0, 32), cg))
                                    mms.append(((64, 128), off + 64 - rl,
                                                (0, 64), cg))
                                else:
                                    mms.append(((rl, 128), off, (0, rl), cg))
                        last = len(mms) - 1
                        for i, ((a, b_), off, tpos, cg) in enumerate(mms):
                            kwargs = {}
                            if tpos is not None:
                                kwargs["tile_position"] = tpos
                            nc.tensor.matmul(
                                av[a:b_, 64 * h:64 * h + 33],
                                lhsT=ptils[gi][:, off:off + (b_ - a)],
                                rhs=vsb[:, 264 * cg + 33 * h:264 * cg + 33 * (h + 1)],
                                start=(i == 0), stop=(i == last),
                                skip_group_check=True,
                                **kwargs)
            def emit_tail(k):
                ptils, packs, av = state.pop(k)
                # normalize
                av3 = av.rearrange("p (h e) -> p h e", e=64)
                zrec = small.tile([128, NUM_HEADS], f32, tag="zrec")
                nc.vector.reciprocal(zrec, av3[:, :, 32])
                zr = zrec[:, :]
                zb = bass.AP(tensor=zr.tensor, offset=zr.offset,
                             ap=[list(zr.ap[0]), [1, NUM_HEADS], [0, 32]])
                ao = small.tile([128, DIM], bf16, tag="ao")
                nc.vector.tensor_mul(
                    ao.rearrange("p (h e) -> p h e", e=32), av3[:, :, 0:32], zb)

                if stage < 4:
                    obuf = obufs[k // 4]
                    nc.vector.tensor_copy(
                        obuf[:, DIM * (k % 4):DIM * (k % 4 + 1)], ao)
                    if k % 4 == 3:
                        k0 = k - 3
                        nc.sync.dma_start(
                            out=out_ext[128 * k0:128 * (k0 + 4), :],
                            in_=obuf.rearrange("p (k d) -> p k d", k=4))
                    return
                # per-tile output chain (transpose-out and proj-out share
                # one PSUM bank: bf16 cols 0:256, f32 via bitcast after)
                combo = psU.tile([128, 1024], bf16, tag="u", name=f"combo{k}")
                tp = combo[:, 0:256]
                for fc in range(2):
                    nc.tensor.transpose(
                        tp[:, 128 * fc:128 * (fc + 1)],
                        ao[:, 128 * fc:128 * (fc + 1)], ident_sb)
                aoT = small.tile([128, DIM], bf16, tag="aoT")
                nc.vector.tensor_copy(aoT, tp)
                pr = combo[:, 512:1024].bitcast(f32)
                for fc in range(2):
                    nc.tensor.matmul(pr, lhsT=aoT[:, 128 * fc:128 * (fc + 1)],
                                     rhs=wproj_sb[fc][:, :],
                                     start=(fc == 0), stop=(fc == 1))
                obuf = obufs[k // 4]
                nc.vector.tensor_add(obuf[:, DIM * (k % 4):DIM * (k % 4 + 1)],
                                     pr, bias_rep)
                if k % 4 == 3:
                    # store 4 tiles in permuted (k, j, t) row order; host
                    # un-permutes to token order.
                    k0 = k - 3
                    nc.sync.dma_start(
                        out=out_ext[128 * k0:128 * (k0 + 4), :],
                        in_=obuf.rearrange("p (k d) -> p k d", k=4))

            if stage >= 2:
                emit_scores(0)
                for k in range(1, ntiles):
                    if 2 <= k <= 14:
                        emit_v(k + 1)
                    for gi, heads in enumerate(GROUPS):
                        emit_scores_group(k, gi)
                        if stage >= 3:
                            emit_av_group(k - 1, gi)
                    if stage >= 3:
                        emit_tail(k - 1)
                if stage >= 3:
                    for gi in range(len(GROUPS)):
                        emit_av_group(ntiles - 1, gi)
                    emit_tail(ntiles - 1)
            if stage < 3:
                for i in range(4):
                    nc.vector.memset(obufs[i], 0.0)
                    nc.sync.dma_start(
                        out=out_ext[512 * i:512 * (i + 1), :],
                        in_=obufs[i].rearrange("p (k d) -> p k d", k=4))
    nc.finalize()
    return nc


def kernel(x, w, Wqkv, Wproj, bproj, **kw):
    global LAST_EXEC_NS, LAST_RES
    assert int(w) == W
    x = np.asarray(x, dtype=np.float32)
    Wqkv = np.asarray(Wqkv, dtype=np.float32)
    Wproj = np.asarray(Wproj, dtype=np.float32)
    bproj = np.asarray(bproj, dtype=np.float32)

    if "prog" not in _CACHE:
        _CACHE["prog"] = _build_program()
        _CACHE["consts"] = _host_consts()
    nc = _CACHE["prog"]
    maskr, oh, ident = _CACHE["consts"]
    wqkvr = _reorder_weights(Wqkv)

    in_maps = []
    wqkvr = wqkvr.astype(bf)
    Wproj_bf = Wproj.astype(bf)
    for b in range(B):
        in_maps.append({
            "x": np.ascontiguousarray(x[b]),
            "wqkvr": wqkvr,
            "wproj": Wproj_bf,
            "bproj": bproj,
            "maskr": maskr,
            "oh": oh,
            "ident": ident,
        })
    res = bass_utils.run_bass_kernel_spmd(nc, in_maps, list(range(B)))
    LAST_RES = res
    LAST_EXEC_NS = res.exec_time_ns
    outs = []
    for b in range(B):
        # store layout: groups of 4 tiles; row within group = (j*16+t)*4 + kk
        perm = np.asarray(res.results[b]["out"]).reshape(4, 8, 16, 4, DIM)
        outs.append(np.transpose(perm, (2, 0, 3, 1, 4)).reshape(N, DIM))
    return np.stack(outs, axis=0).astype(np.float32)


# revision 5
# speedup vs baseline: 1.0310x; 1.0310x over previous
"""Trainium2 Bass kernel v2 for nn_Attention_39865886442202 (periodic local attention).

Design (per core, one batch element):
  - Mask folded into the QK matmul: K-ext tiles carry 8 onehot rows, Q-ext tiles
    carry 8 mask-value rows (3 variants for chunk offsets ci in {-1,0,+1}),
    giving a single 40-row-contraction matmul per (tile, chunk, head).
  - 2 heads per 128-partition "pair" tile at slots {rows 0-40, rows 64-104};
    QKV projection uses zero-padded weight columns so PSUM rows land directly
    on the slots (one evacuation copy per (nf, pair-block)).
  - Query-column windows per chunk: center full 128; e- cols [0, 16*jhi);
    e+ cols [16*jlo2, 128) with jlo2 even (AV partition-alignment rules).
  - Scores for 4 heads packed into one 3-bank PSUM tile; ONE exp per group.
  - AV with interleaved ones-column in V gives softmax denominators.
  - Per-tile output chain: transpose -> project -> bias -> scattered DMA store.
"""

import numpy as np
import ml_dtypes

import concourse.bass as bass
import concourse.mybir as mybir
import concourse.tile as tile
from concourse import bacc, bass_utils

DIM = 256
NUM_HEADS = 8
HEAD_DIM = 32
SCALE = HEAD_DIM ** (-0.5)
W_BUF = 160
WK = 11
B = 8
N = 2048
W = 128
T = N // W         # 16 token blocks
NTILES = 16        # query tiles (8 residues x 16 t)
NEG = -30000.0

bf = ml_dtypes.bfloat16
f32 = mybir.dt.float32
bf16 = mybir.dt.bfloat16

_CACHE = {}
LAST_EXEC_NS = None
LAST_RES = None


def _build_m1(w):
    Wb = W_BUF
    mask = np.ones((Wb, Wb), dtype=np.float32)
    for i in range(Wb):
        b = i - WK // 2 if i - WK // 2 > 0 else 0
        if b > Wb - WK:
            b = Wb - WK
        mask[i, b:b + WK] = 0.0
    mask[mask >= 1] = -np.inf
    off = (Wb - w) // 2
    m1 = mask[off:Wb - off, off:Wb - off].copy()
    e = WK // 2 + 1
    m1[:e] = mask[:e, :w]
    m1[-e:] = mask[-e:, -w:]
    return m1  # [128, 128], 0 = visible, -inf = masked


def _windows():
    """Per (tile k): ordered blocks [(cg, jlo2, jhi)] — center first."""
    m1 = _build_m1(W)
    wins = {}
    for k in range(NTILES):
        blocks = []
        for cg in (k, k - 1, k + 1):
            if not (0 <= cg < NTILES):
                continue
            js = [j for j in range(8)
                  if (m1[8 * k + j, 8 * cg:8 * cg + 8] == 0).any()]
            assert js == list(range(min(js), max(js) + 1))
            jlo, jhi = min(js), max(js) + 1
            if cg == k:
                assert (jlo, jhi) == (0, 8)
            if cg > k:
                assert jhi == 8
                jlo = jlo - (jlo % 2)
            if cg < k:
                assert jlo == 0
            blocks.append((cg, jlo, jhi))
        wins[k] = blocks
    return wins


def _host_consts():
    m1 = _build_m1(W)
    m1neg = np.where(m1 == 0.0, 0.0, NEG).astype(np.float32)
    # mask rows: maskr[v][r, p*16 + t] = m1[p, 8*(p//8 + v - 1) + r]
    maskr = np.full((3, 8, N), NEG, dtype=np.float32)
    for v in range(3):
        ci = v - 1
        for p in range(W):
            cg = p // 8 + ci
            if not (0 <= cg < NTILES):
                continue
            for r in range(8):
                maskr[v, r, p * 16:(p + 1) * 16] = m1neg[p, 8 * cg + r]
    # onehot rows: oh[r, c*16 + s] = 1 iff c % 8 == r
    oh = np.zeros((8, N), dtype=np.float32)
    for c in range(W):
        oh[c % 8, c * 16:(c + 1) * 16] = 1.0
    ident = np.eye(128, dtype=np.float32)
    return (maskr.reshape(24, N).astype(bf), oh.astype(bf), ident.astype(bf))


def _reorder_weights(Wqkv):
    """[256, 1280]: 4 padded q-pair blocks, 4 padded k-pair blocks, V."""
    Wq = Wqkv[:, :DIM] * SCALE
    Wk = Wqkv[:, DIM:2 * DIM]
    Wv = Wqkv[:, 2 * DIM:]
    out = np.zeros((DIM, 1280), dtype=np.float32)
    for g in range(4):
        out[:, 128 * g + 0:128 * g + 32] = Wq[:, 32 * (2 * g):32 * (2 * g + 1)]
        out[:, 128 * g + 64:128 * g + 96] = Wq[:, 32 * (2 * g + 1):32 * (2 * g + 2)]
        out[:, 512 + 128 * g + 0:512 + 128 * g + 32] = Wk[:, 32 * (2 * g):32 * (2 * g + 1)]
        out[:, 512 + 128 * g + 64:512 + 128 * g + 96] = Wk[:, 32 * (2 * g + 1):32 * (2 * g + 2)]
    out[:, 1024:1280] = Wv
    return out


def _pack_groups(blocks, heads):
    """Pack per-head block lists into one <=1024-col (2-bank) PSUM tile.
    HW constraint: matmuls sharing a PSUM bank must share a PE row-tile, so a
    bank may only hold blocks of one slot (head parity); bump to the next bank
    on slot change. Returns ([(h_idx, cg, jlo, jhi, off, width)], [runs])."""
    out = []
    runs = []
    off = 0
    bank_slot = {}
    for h_idx, h in enumerate(heads):
        sl = h % 2
        for (cg, jlo, jhi) in blocks:
            width = 16 * (jhi - jlo)
            if off // 512 != (off + width - 1) // 512:
                off = (off // 512 + 1) * 512  # bump past bank boundary
            if bank_slot.get(off // 512, sl) != sl:
                off = (off // 512 + 1) * 512  # bump on slot change
            bank_slot[off // 512] = sl
            out.append((h_idx, cg, jlo, jhi, off, width))
            if runs and runs[-1][1] == off:
                runs[-1] = (runs[-1][0], off + width)
            else:
                runs.append((off, off + width))
            off += width
    assert off <= 1024, off
    return out, runs


def _build_program(stage=4, ntiles=NTILES, noexp=False, nblocks=None):
    wins = _windows()
    nc = bacc.Bacc(None, target_bir_lowering=False)

    x_in = nc.declare_dram_parameter("x", [N, DIM], f32, isOutput=False)
    wqkvr_in = nc.declare_dram_parameter("wqkvr", [DIM, 1280], bf16, isOutput=False)
    wproj_in = nc.declare_dram_parameter("wproj", [DIM, DIM], bf16, isOutput=False)
    bproj_in = nc.declare_dram_parameter("bproj", [DIM], f32, isOutput=False)
    maskr_in = nc.declare_dram_parameter("maskr", [24, N], bf16, isOutput=False)
    oh_in = nc.declare_dram_parameter("oh", [8, N], bf16, isOutput=False)
    ident_in = nc.declare_dram_parameter("ident", [128, 128], bf16, isOutput=False)
    out_ext = nc.declare_dram_parameter("out", [N, DIM], bf16, isOutput=True)

    with tile.TileContext(nc) as tc:
        with (
            tc.tile_pool(name="sing", bufs=1) as sing,
            tc.tile_pool(name="sbp", bufs=2) as sbp,
            tc.tile_pool(name="small", bufs=4) as small,
            tc.tile_pool(name="psS", bufs=3, space="PSUM") as psS,
            tc.tile_pool(name="psU", bufs=2, space="PSUM") as psU,
        ):
            # ---- act table preload ----
            scratch = small.tile([128, 1], f32, tag="scr")
            nc.vector.memset(scratch, 0.0)
            warm = small.tile([128, 1], f32, tag="scr2")
            nc.scalar.activation(warm, scratch, mybir.ActivationFunctionType.Exp)

            # ---- x load first (the long pole), 4 chunks ----
            xbf = sing.tile([128, T * DIM], bf16)
            xin = x_in.rearrange("(t p) d -> p t d", p=128)
            xbf3 = xbf.rearrange("p (t d) -> p t d", t=T)
            for t4 in range(4):
                nc.gpsimd.dma_start(out=xbf3[:, 4 * t4:4 * (t4 + 1), :],
                                    in_=xin[:, 4 * t4:4 * (t4 + 1), :])
            ident_sb = sing.tile([128, 128], bf16)
            nc.sync.dma_start(out=ident_sb, in_=ident_in[:, :])
            wqkv_sb = []
            for dc in range(2):
                t_ = sing.tile([128, 1280], bf16, tag=f"wqkv{dc}")
                nc.sync.dma_start(out=t_, in_=wqkvr_in[128 * dc:128 * (dc + 1), :])
                wqkv_sb.append(t_)
            wproj_sb = []
            for dc in range(2):
                t_ = sing.tile([128, DIM], bf16, tag=f"wproj{dc}")
                nc.sync.dma_start(out=t_, in_=wproj_in[128 * dc:128 * (dc + 1), :])
                wproj_sb.append(t_)
            bias_rep = sing.tile([128, DIM], f32)
            bp = bproj_in[:]
            bproj_bcast = bass.AP(tensor=bp.tensor, offset=bp.offset,
                                  ap=[[0, 128], [1, DIM]])
            nc.gpsimd.dma_start(out=bias_rep, in_=bproj_bcast)
            # xTg: residue-major x^T (col = c*16 + s, token n = s*128 + c),
            # written directly by scattering the transpose evacuations.
            xTg = [sing.tile([128, N], bf16, name=f"xTg{dc}", tag=f"xTg{dc}")
                   for dc in range(2)]
            for t4 in range(4):
                for dc in range(2):
                    tp = psU.tile([128, 512], bf16, tag="u")
                    for i in range(4):
                        t = 4 * t4 + i
                        nc.tensor.transpose(
                            tp[:, 128 * i:128 * (i + 1)],
                            xbf[:, 256 * t + 128 * dc:256 * t + 128 * (dc + 1)],
                            ident_sb)
                    dt_ = xTg[dc][:, :]
                    dst = bass.AP(tensor=dt_.tensor, offset=dt_.offset + 4 * t4,
                                  ap=[list(dt_.ap[0]), [1, 4], [16, 128]])
                    sv = tp[:, :]
                    src = bass.AP(tensor=sv.tensor, offset=sv.offset,
                                  ap=[list(sv.ap[0]), [128, 4], [1, 128]])
                    nc.vector.tensor_copy(dst, src)

            # ---- QK projection into Qext/Kext pair tiles (per-pair chain:
            # staging copies -> v1 mask + onehot rows -> replication -> v0/v2
            # mask rows), so attention unblocks pair by pair ----
            qext = [sing.tile([128, 4 * N], bf16, name=f"qext{v}", tag=f"qext{v}") for v in range(3)]
            kext = sing.tile([128, 4 * N], bf16)

            def bcast_rows(dst_tile, r0, nrows, src_rows, c0, npair):
                dv = dst_tile[r0:r0 + nrows, 2048 * c0:2048 * (c0 + npair)]
                dst = bass.AP(tensor=dv.tensor, offset=dv.offset,
                              ap=[list(dv.ap[0]), [N, npair], [1, N]])
                sv = src_rows
                src = bass.AP(tensor=sv.tensor, offset=sv.offset,
                              ap=[list(sv.ap[0]), [0, npair], [1, N]])
                nc.sync.dma_start(out=dst, in_=src)

            # slot-1 mask/onehot rows preloaded (staging leaves rows 96+ alone)
            for v in range(3):
                bcast_rows(qext[v], 96, 8, maskr_in[8 * v:8 * v + 8, :], 0, 4)
            bcast_rows(kext, 96, 8, oh_in[:, :], 0, 4)

            for g in range(4):        # pair index
                for qk in range(2):   # 0 = q, 1 = k
                    blk = 4 * qk + g
                    for nf2 in range(2):
                        ps = psS.tile([128, 1024], f32, tag="sc",
                                      name=f"qkps{blk}{nf2}")
                        for half in range(2):
                            nf = 2 * nf2 + half
                            for dc in range(2):
                                nc.tensor.matmul(
                                    ps[:, 512 * half:512 * (half + 1)],
                                    lhsT=wqkv_sb[dc][:, 128 * blk:128 * (blk + 1)],
                                    rhs=xTg[dc][:, 512 * nf:512 * (nf + 1)],
                                    start=(dc == 0), stop=(dc == 1),
                                )
                        # contiguous evac (xTg order == qext/kext col order);
                        # rows [0:96] only so slot-1 mask/onehot rows survive
                        dtile = qext[1] if qk == 0 else kext
                        dst = dtile[0:96, 2048 * g + 1024 * nf2:
                                    2048 * g + 1024 * (nf2 + 1)]
                        if qk == 0:
                            nc.scalar.copy(dst, ps[0:96, :])
                        else:
                            nc.vector.tensor_copy(dst, ps[0:96, :])
                if g in (2, 3):
                    # chains cover pairs 0-2 (heads 0-5) then pair 3 (heads 6,7)
                    c0, npair = (0, 3) if g == 2 else (3, 1)
                    for v in range(3):
                        bcast_rows(qext[v], 32, 8, maskr_in[8 * v:8 * v + 8, :],
                                   c0, npair)
                    bcast_rows(kext, 32, 8, oh_in[:, :], c0, npair)
                    for v in (0, 2):
                        for r0 in (0, 64):
                            nc.sync.dma_start(
                                out=qext[v][r0:r0 + 32,
                                            2048 * c0:2048 * (c0 + npair)],
                                in_=qext[1][r0:r0 + 32,
                                            2048 * c0:2048 * (c0 + npair)])
            # ---- V projection (keys c-major), interleaved ones column ----
            vsb = sing.tile([128, 16 * 264], bf16)
            vsb4 = vsb.rearrange("p (m h e) -> p m h e", m=16, e=33)
            nc.vector.memset(vsb4[:, :, :, 32:33], 1.0)

            def emit_v(m):
                ps = psS.tile([128, DIM], f32, tag="sc", name=f"vps{m}")
                for dc in range(2):
                    nc.tensor.matmul(
                        ps,
                        lhsT=xTg[dc][:, 128 * m:128 * (m + 1)],
                        rhs=wqkv_sb[dc][:, 1024:1280],
                        start=(dc == 0), stop=(dc == 1),
                    )
                nc.vector.tensor_copy(
                    vsb4[:, m, :, 0:32],
                    ps.rearrange("p (h e) -> p h e", h=NUM_HEADS),
                )

            for m in range(3):
                emit_v(m)

            # ---- attention (one-tile software pipeline) ----
            # heads grouped by slot: matmuls sharing a PSUM bank must
            # use the same PE row-tile (hw constraint)
            GROUPS = [(6, 7), (0, 2, 4), (1, 3, 5)]
            state = {}
            obufs = [sing.tile([128, 4 * DIM], bf16, name=f"obuf{i}",
                               tag=f"obuf{i}") for i in range(4)]

            def emit_scores_group(k, gi):
                heads = GROUPS[gi]
                pack, runs = _pack_groups(wins[k], heads)
                if nblocks is not None:
                    pack = pack[:nblocks]
                    runs = runs[:1]
                sps = psS.tile([128, 1024], f32, tag="sc", name=f"sps{k}g{gi}")
                for (hh, cg, jlo, jhi, off, width) in pack:
                    h = heads[hh]
                    g, sl = h // 2, h % 2
                    v = cg - k + 1
                    lhsT = kext[64 * sl:64 * sl + 40,
                                2048 * g + 128 * cg:2048 * g + 128 * (cg + 1)]
                    rhs = qext[v][64 * sl:64 * sl + 40,
                                  2048 * g + 128 * k + 16 * jlo:
                                  2048 * g + 128 * k + 16 * jhi]
                    nc.tensor.matmul(sps[:, off:off + width], lhsT=lhsT,
                                     rhs=rhs, start=True, stop=True)
                ptil = sbp.tile([128, 1024], bf16, tag="ptil", bufs=6, name=f"pt{k}g{gi}")
                for (a, b_) in runs:
                    if noexp:
                        nc.vector.tensor_copy(ptil[:, a:b_], sps[:, a:b_])
                    else:
                        nc.scalar.activation(ptil[:, a:b_], sps[:, a:b_],
                                             mybir.ActivationFunctionType.Exp)
                if k not in state:
                    state[k] = ([], [], None)
                state[k][0].append(ptil)
                state[k][1].append(pack)

            def emit_scores(k):
                for gi in range(len(GROUPS)):
                    emit_scores_group(k, gi)

            def emit_av_group(k, gi):
                ptils, packs, av = state[k][:3]
                if av is None:
                    av = psU.tile([128, 512], f32, tag="u", name=f"av{k}")
                    state[k] = (ptils, packs, av)
                heads = GROUPS[gi]
                if True:
                    for hh, h in enumerate(heads):
                        blocks = [b for b in packs[gi] if b[0] == hh]
                        mms = []
                        for (bh, cg, jlo, jhi, off, width) in blocks:
                            if jlo == 0:
                                mms.append(((0, 16 * jhi), off, None, cg))
                            else:
                                rl = 16 * jlo
                                if rl < 64:
                                    mms.append(((rl, 64), off, (0, 32), cg))
                                    mms.append(((64, 128), off + 64 - rl,
                                                (0, 64), cg))
                                else:
                                    mms.append(((rl, 128), off, (0, rl), cg))
                        last = len(mms) - 1
                        for i, ((a, b_), off, tpos, cg) in enumerate(mms):
                            kwargs = {}
                            if tpos is not None:
                                kwargs["tile_position"] = tpos
                            nc.tensor.matmul(
                                av[a:b_, 64 * h:64 * h + 33],
                                lhsT=ptils[gi][:, off:off + (b_ - a)],
                                rhs=vsb[:, 264 * cg + 33 * h:264 * cg + 33 * (h + 1)],
                                start=(i == 0), stop=(i == last),
                                skip_group_check=True,
                                **kwargs)
            def emit_tail(k):
                ptils, packs, av = state.pop(k)
                # normalize
                av3 = av.rearrange("p (h e) -> p h e", e=64)
                zrec = small.tile([128, NUM_HEADS], f32, tag="zrec")
                nc.vector.reciprocal(zrec, av3[:, :, 32])
                zr = zrec[:, :]
                zb = bass.AP(tensor=zr.tensor, offset=zr.offset,
                             ap=[list(zr.ap[0]), [1, NUM_HEADS], [0, 32]])
                ao = small.tile([128, DIM], bf16, tag="ao")
                nc.vector.tensor_mul(
                    ao.rearrange("p (h e) -> p h e", e=32), av3[:, :, 0:32], zb)

                if stage < 4:
                    obuf = obufs[k // 4]
                    nc.vector.tensor_copy(
                        obuf[:, DIM * (k % 4):DIM * (k % 4 + 1)], ao)
                    if k % 4 == 3:
                        k0 = k - 3
                        nc.sync.dma_start(
                            out=out_ext[128 * k0:128 * (k0 + 4), :],
                            in_=obuf.rearrange("p (k d) -> p k d", k=4))
                    return
                # per-tile output chain (transpose-out and proj-out share
                # one PSUM bank: bf16 cols 0:256, f32 via bitcast after)
                combo = psU.tile([128, 1024], bf16, tag="u", name=f"combo{k}")
                tp = combo[:, 0:256]
                for fc in range(2):
                    nc.tensor.transpose(
                        tp[:, 128 * fc:128 * (fc + 1)],
                        ao[:, 128 * fc:128 * (fc + 1)], ident_sb)
                aoT = small.tile([128, DIM], bf16, tag="aoT")
                nc.vector.tensor_copy(aoT, tp)
                pr = combo[:, 512:1024].bitcast(f32)
                for fc in range(2):
                    nc.tensor.matmul(pr, lhsT=aoT[:, 128 * fc:128 * (fc + 1)],
                                     rhs=wproj_sb[fc][:, :],
                                     start=(fc == 0), stop=(fc == 1))
                obuf = obufs[k // 4]
                nc.vector.tensor_add(obuf[:, DIM * (k % 4):DIM * (k % 4 + 1)],
                                     pr, bias_rep)
                if k % 4 == 3:
                    # store 4 tiles in permuted (k, j, t) row order; host
                    # un-permutes to token order.
                    k0 = k - 3
                    nc.sync.dma_start(
                        out=out_ext[128 * k0:128 * (k0 + 4), :],
                        in_=obuf.rearrange("p (k d) -> p k d", k=4))

            if stage >= 2:
                emit_scores(0)
                for k in range(1, ntiles):
                    if 2 <= k <= 14:
                        emit_v(k + 1)
                    for gi, heads in enumerate(GROUPS):
                        emit_scores_group(k, gi)
                        if stage >= 3:
                            emit_av_group(k - 1, gi)
                    if stage >= 3:
                        emit_tail(k - 1)
                if stage >= 3:
                    for gi in range(len(GROUPS)):
                        emit_av_group(ntiles - 1, gi)
                    emit_tail(ntiles - 1)
            if stage < 3:
                for i in range(4):
                    nc.vector.memset(obufs[i], 0.0)
                    nc.sync.dma_start(
                        out=out_ext[512 * i:512 * (i + 1), :],
                        in_=obufs[i].rearrange("p (k d) -> p k d", k=4))
    nc.finalize()
    return nc


def kernel(x, w, Wqkv, Wproj, bproj, **kw):
    global LAST_EXEC_NS, LAST_RES
    assert int(w) == W
    x = np.asarray(x, dtype=np.float32)
    Wqkv = np.asarray(Wqkv, dtype=np.float32)
    Wproj = np.asarray(Wproj, dtype=np.float32)
    bproj = np.asarray(bproj, dtype=np.float32)

    if "prog" not in _CACHE:
        _CACHE["prog"] = _build_program()
        _CACHE["consts"] = _host_consts()
    nc = _CACHE["prog"]
    maskr, oh, ident = _CACHE["consts"]
    wqkvr = _reorder_weights(Wqkv)

    in_maps = []
    wqkvr = wqkvr.astype(bf)
    Wproj_bf = Wproj.astype(bf)
    for b in range(B):
        in_maps.append({
            "x": np.ascontiguousarray(x[b]),
            "wqkvr": wqkvr,
            "wproj": Wproj_bf,
            "bproj": bproj,
            "maskr": maskr,
            "oh": oh,
            "ident": ident,
        })
    res = bass_utils.run_bass_kernel_spmd(nc, in_maps, list(range(B)))
    LAST_RES = res
    LAST_EXEC_NS = res.exec_time_ns
    outs = []
    for b in range(B):
        # store layout: groups of 4 tiles; row within group = (j*16+t)*4 + kk
        perm = np.asarray(res.results[b]["out"]).reshape(4, 8, 16, 4, DIM)
        outs.append(np.transpose(perm, (2, 0, 3, 1, 4)).reshape(N, DIM))
    return np.stack(outs, axis=0).astype(np.float32)


# revision 10
# speedup vs baseline: 1.0691x; 1.0369x over previous
"""Trainium2 Bass kernel v2 for nn_Attention_39865886442202 (periodic local attention).

Design (per core, one batch element):
  - Mask folded into the QK matmul: K-ext tiles carry 8 onehot rows, Q-ext tiles
    carry 8 mask-value rows (3 variants for chunk offsets ci in {-1,0,+1}),
    giving a single 40-row-contraction matmul per (tile, chunk, head).
  - 2 heads per 128-partition "pair" tile at slots {rows 0-40, rows 64-104};
    QKV projection uses zero-padded weight columns so PSUM rows land directly
    on the slots (one evacuation copy per (nf, pair-block)).
  - Query-column windows per chunk: center full 128; e- cols [0, 16*jhi);
    e+ cols [16*jlo2, 128) with jlo2 even (AV partition-alignment rules).
  - Scores for 4 heads packed into one 3-bank PSUM tile; ONE exp per group.
  - AV with interleaved ones-column in V gives softmax denominators.
  - Per-tile output chain: transpose -> project -> bias -> scattered DMA store.
"""

import numpy as np
import ml_dtypes

import concourse.bass as bass
import concourse.mybir as mybir
import concourse.tile as tile
from concourse import bacc, bass_utils

DIM = 256
NUM_HEADS = 8
HEAD_DIM = 32
SCALE = HEAD_DIM ** (-0.5)
W_BUF = 160
WK = 11
B = 8
N = 2048
W = 128
T = N // W         # 16 token blocks
NTILES = 16        # query tiles (8 residues x 16 t)
NEG = -30000.0

bf = ml_dtypes.bfloat16
f32 = mybir.dt.float32
bf16 = mybir.dt.bfloat16

_CACHE = {}
LAST_EXEC_NS = None
LAST_RES = None


def _build_m1(w):
    Wb = W_BUF
    mask = np.ones((Wb, Wb), dtype=np.float32)
    for i in range(Wb):
        b = i - WK // 2 if i - WK // 2 > 0 else 0
        if b > Wb - WK:
            b = Wb - WK
        mask[i, b:b + WK] = 0.0
    mask[mask >= 1] = -np.inf
    off = (Wb - w) // 2
    m1 = mask[off:Wb - off, off:Wb - off].copy()
    e = WK // 2 + 1
    m1[:e] = mask[:e, :w]
    m1[-e:] = mask[-e:, -w:]
    return m1  # [128, 128], 0 = visible, -inf = masked


def _windows():
    """Per (tile k): ordered blocks [(cg, jlo2, jhi)] — center first."""
    m1 = _build_m1(W)
    wins = {}
    for k in range(NTILES):
        blocks = []
        for cg in (k, k - 1, k + 1):
            if not (0 <= cg < NTILES):
                continue
            js = [j for j in range(8)
                  if (m1[8 * k + j, 8 * cg:8 * cg + 8] == 0).any()]
            assert js == list(range(min(js), max(js) + 1))
            jlo, jhi = min(js), max(js) + 1
            if cg == k:
                assert (jlo, jhi) == (0, 8)
            if cg > k:
                assert jhi == 8
                jlo = jlo - (jlo % 2)
            if cg < k:
                assert jlo == 0
            blocks.append((cg, jlo, jhi))
        wins[k] = blocks
    return wins


def _host_consts():
    m1 = _build_m1(W)
    m1neg = np.where(m1 == 0.0, 0.0, NEG).astype(np.float32)
    # mask rows: maskr[v][r, p*16 + t] = m1[p, 8*(p//8 + v - 1) + r]
    maskr = np.full((3, 8, N), NEG, dtype=np.float32)
    for v in range(3):
        ci = v - 1
        for p in range(W):
            cg = p // 8 + ci
            if not (0 <= cg < NTILES):
                continue
            for r in range(8):
                maskr[v, r, p * 16:(p + 1) * 16] = m1neg[p, 8 * cg + r]
    # onehot rows: oh[r, c*16 + s] = 1 iff c % 8 == r
    oh = np.zeros((8, N), dtype=np.float32)
    for c in range(W):
        oh[c % 8, c * 16:(c + 1) * 16] = 1.0
    ident = np.eye(128, dtype=np.float32)
    return (maskr.reshape(24, N).astype(bf), oh.astype(bf), ident.astype(bf))


def _reorder_weights(Wqkv):
    """[256, 1280]: 4 padded q-pair blocks, 4 padded k-pair blocks, V."""
    Wq = Wqkv[:, :DIM] * SCALE
    Wk = Wqkv[:, DIM:2 * DIM]
    Wv = Wqkv[:, 2 * DIM:]
    out = np.zeros((DIM, 1280), dtype=np.float32)
    for g in range(4):
        out[:, 128 * g + 0:128 * g + 32] = Wq[:, 32 * (2 * g):32 * (2 * g + 1)]
        out[:, 128 * g + 64:128 * g + 96] = Wq[:, 32 * (2 * g + 1):32 * (2 * g + 2)]
        out[:, 512 + 128 * g + 0:512 + 128 * g + 32] = Wk[:, 32 * (2 * g):32 * (2 * g + 1)]
        out[:, 512 + 128 * g + 64:512 + 128 * g + 96] = Wk[:, 32 * (2 * g + 1):32 * (2 * g + 2)]
    out[:, 1024:1280] = Wv
    return out


def _pack_groups(blocks, heads):
    """Pack per-head block lists into one <=1024-col (2-bank) PSUM tile.
    HW constraint: matmuls sharing a PSUM bank must share a PE row-tile, so a
    bank may only hold blocks of one slot (head parity); bump to the next bank
    on slot change. Returns ([(h_idx, cg, jlo, jhi, off, width)], [runs])."""
    out = []
    runs = []
    off = 0
    bank_slot = {}
    for h_idx, h in enumerate(heads):
        sl = h % 2
        for (cg, jlo, jhi) in blocks:
            width = 16 * (jhi - jlo)
            if off // 512 != (off + width - 1) // 512:
                off = (off // 512 + 1) * 512  # bump past bank boundary
            if bank_slot.get(off // 512, sl) != sl:
                off = (off // 512 + 1) * 512  # bump on slot change
            bank_slot[off // 512] = sl
            out.append((h_idx, cg, jlo, jhi, off, width))
            if runs and runs[-1][1] == off:
                runs[-1] = (runs[-1][0], off + width)
            else:
                runs.append((off, off + width))
            off += width
    assert off <= 1024, off
    return out, runs


def _build_program(stage=4, ntiles=NTILES, noexp=False, nblocks=None):
    wins = _windows()
    nc = bacc.Bacc(None, target_bir_lowering=False)

    x_in = nc.declare_dram_parameter("x", [N, DIM], f32, isOutput=False)
    wqkvr_in = nc.declare_dram_parameter("wqkvr", [DIM, 1280], bf16, isOutput=False)
    wproj_in = nc.declare_dram_parameter("wproj", [DIM, DIM], bf16, isOutput=False)
    bproj_in = nc.declare_dram_parameter("bproj", [DIM], f32, isOutput=False)
    maskr_in = nc.declare_dram_parameter("maskr", [24, N], bf16, isOutput=False)
    oh_in = nc.declare_dram_parameter("oh", [8, N], bf16, isOutput=False)
    ident_in = nc.declare_dram_parameter("ident", [128, 128], bf16, isOutput=False)
    out_ext = nc.declare_dram_parameter("out", [N, DIM], bf16, isOutput=True)

    with tile.TileContext(nc) as tc:
        with (
            tc.tile_pool(name="sing", bufs=1) as sing,
            tc.tile_pool(name="sbp", bufs=2) as sbp,
            tc.tile_pool(name="small", bufs=4) as small,
            tc.tile_pool(name="psS", bufs=3, space="PSUM") as psS,
            tc.tile_pool(name="psU", bufs=2, space="PSUM") as psU,
        ):
            # ---- act table preload ----
            scratch = small.tile([128, 1], f32, tag="scr")
            nc.vector.memset(scratch, 0.0)
            warm = small.tile([128, 1], f32, tag="scr2")
            nc.scalar.activation(warm, scratch, mybir.ActivationFunctionType.Exp)

            # ---- x load first (the long pole), 4 chunks ----
            xbf = sing.tile([128, T * DIM], bf16)
            xin = x_in.rearrange("(t p) d -> p t d", p=128)
            xbf3 = xbf.rearrange("p (t d) -> p t d", t=T)
            for t4 in range(4):
                nc.gpsimd.dma_start(out=xbf3[:, 4 * t4:4 * (t4 + 1), :],
                                    in_=xin[:, 4 * t4:4 * (t4 + 1), :])
            ident_sb = sing.tile([128, 128], bf16)
            nc.sync.dma_start(out=ident_sb, in_=ident_in[:, :])
            wqkv_sb = []
            for dc in range(2):
                t_ = sing.tile([128, 1280], bf16, tag=f"wqkv{dc}")
                nc.sync.dma_start(out=t_, in_=wqkvr_in[128 * dc:128 * (dc + 1), :])
                wqkv_sb.append(t_)
            wproj_sb = []
            for dc in range(2):
                t_ = sing.tile([128, DIM], bf16, tag=f"wproj{dc}")
                nc.sync.dma_start(out=t_, in_=wproj_in[128 * dc:128 * (dc + 1), :])
                wproj_sb.append(t_)
            bias_rep = sing.tile([128, DIM], f32)
            bp = bproj_in[:]
            bproj_bcast = bass.AP(tensor=bp.tensor, offset=bp.offset,
                                  ap=[[0, 128], [1, DIM]])
            nc.gpsimd.dma_start(out=bias_rep, in_=bproj_bcast)
            # xTg: residue-major x^T (col = c*16 + s, token n = s*128 + c),
            # written directly by scattering the transpose evacuations.
            xTg = [sing.tile([128, N], bf16, name=f"xTg{dc}", tag=f"xTg{dc}")
                   for dc in range(2)]
            for t4 in range(4):
                for dc in range(2):
                    tp = psU.tile([128, 512], bf16, tag="u")
                    for i in range(4):
                        t = 4 * t4 + i
                        nc.tensor.transpose(
                            tp[:, 128 * i:128 * (i + 1)],
                            xbf[:, 256 * t + 128 * dc:256 * t + 128 * (dc + 1)],
                            ident_sb)
                    dt_ = xTg[dc][:, :]
                    dst = bass.AP(tensor=dt_.tensor, offset=dt_.offset + 4 * t4,
                                  ap=[list(dt_.ap[0]), [1, 4], [16, 128]])
                    sv = tp[:, :]
                    src = bass.AP(tensor=sv.tensor, offset=sv.offset,
                                  ap=[list(sv.ap[0]), [128, 4], [1, 128]])
                    nc.scalar.copy(dst, src)

            # ---- QK projection into Qext/Kext pair tiles (per-pair chain:
            # staging copies -> v1 mask + onehot rows -> replication -> v0/v2
            # mask rows), so attention unblocks pair by pair ----
            qext = [sing.tile([128, 4 * N], bf16, name=f"qext{v}", tag=f"qext{v}") for v in range(3)]
            kext = sing.tile([128, 4 * N], bf16)

            def bcast_rows(dst_tile, r0, nrows, src_rows, c0, npair):
                dv = dst_tile[r0:r0 + nrows, 2048 * c0:2048 * (c0 + npair)]
                dst = bass.AP(tensor=dv.tensor, offset=dv.offset,
                              ap=[list(dv.ap[0]), [N, npair], [1, N]])
                sv = src_rows
                src = bass.AP(tensor=sv.tensor, offset=sv.offset,
                              ap=[list(sv.ap[0]), [0, npair], [1, N]])
                nc.sync.dma_start(out=dst, in_=src)

            # slot-1 mask/onehot rows preloaded (staging leaves rows 96+ alone)
            for v in range(3):
                bcast_rows(qext[v], 96, 8, maskr_in[8 * v:8 * v + 8, :], 0, 4)
            bcast_rows(kext, 96, 8, oh_in[:, :], 0, 4)

            for g in range(4):        # pair index
                for qk in range(2):   # 0 = q, 1 = k
                    blk = 4 * qk + g
                    for nf2 in range(2):
                        ps = psS.tile([128, 1024], f32, tag="sc",
                                      name=f"qkps{blk}{nf2}")
                        for half in range(2):
                            nf = 2 * nf2 + half
                            for dc in range(2):
                                nc.tensor.matmul(
                                    ps[:, 512 * half:512 * (half + 1)],
                                    lhsT=wqkv_sb[dc][:, 128 * blk:128 * (blk + 1)],
                                    rhs=xTg[dc][:, 512 * nf:512 * (nf + 1)],
                                    start=(dc == 0), stop=(dc == 1),
                                )
                        # contiguous evac (xTg order == qext/kext col order);
                        # rows [0:96] only so slot-1 mask/onehot rows survive
                        dtile = qext[1] if qk == 0 else kext
                        dst = dtile[0:96, 2048 * g + 1024 * nf2:
                                    2048 * g + 1024 * (nf2 + 1)]
                        if qk == 0:
                            nc.scalar.copy(dst, ps[0:96, :])
                        else:
                            nc.vector.tensor_copy(dst, ps[0:96, :])
                if g in (1, 3):
                    # per-half chains: pairs {0,1} then {2,3}
                    c0, npair = (g - 1, 2)
                    for v in range(3):
                        bcast_rows(qext[v], 32, 8, maskr_in[8 * v:8 * v + 8, :],
                                   c0, npair)
                    bcast_rows(kext, 32, 8, oh_in[:, :], c0, npair)
                    for v in (0, 2):
                        for r0 in (0, 64):
                            nc.sync.dma_start(
                                out=qext[v][r0:r0 + 32,
                                            2048 * c0:2048 * (c0 + npair)],
                                in_=qext[1][r0:r0 + 32,
                                            2048 * c0:2048 * (c0 + npair)])
            # ---- V projection (keys c-major), interleaved ones column ----
            vsb = sing.tile([128, 16 * 264], bf16)
            vsb4 = vsb.rearrange("p (m h e) -> p m h e", m=16, e=33)
            nc.vector.memset(vsb4[:, :, :, 32:33], 1.0)

            def emit_v(m):
                ps = psS.tile([128, DIM], f32, tag="sc", name=f"vps{m}")
                for dc in range(2):
                    nc.tensor.matmul(
                        ps,
                        lhsT=xTg[dc][:, 128 * m:128 * (m + 1)],
                        rhs=wqkv_sb[dc][:, 1024:1280],
                        start=(dc == 0), stop=(dc == 1),
                    )
                nc.vector.tensor_copy(
                    vsb4[:, m, :, 0:32],
                    ps.rearrange("p (h e) -> p h e", h=NUM_HEADS),
                )

            for m in range(3):
                emit_v(m)

            # ---- attention (one-tile software pipeline) ----
            # heads grouped by slot: matmuls sharing a PSUM bank must
            # use the same PE row-tile (hw constraint)
            GROUPS = [(0, 2), (1, 3), (4, 6), (5, 7)]
            state = {}
            obufs = [sing.tile([128, 4 * DIM], bf16, name=f"obuf{i}",
                               tag=f"obuf{i}") for i in range(4)]

            def emit_scores_group(k, gi):
                heads = GROUPS[gi]
                pack, runs = _pack_groups(wins[k], heads)
                if nblocks is not None:
                    pack = pack[:nblocks]
                    runs = runs[:1]
                sps = psS.tile([128, 1024], f32, tag="sc", name=f"sps{k}g{gi}")
                for (hh, cg, jlo, jhi, off, width) in pack:
                    h = heads[hh]
                    g, sl = h // 2, h % 2
                    v = cg - k + 1
                    lhsT = kext[64 * sl:64 * sl + 40,
                                2048 * g + 128 * cg:2048 * g + 128 * (cg + 1)]
                    rhs = qext[v][64 * sl:64 * sl + 40,
                                  2048 * g + 128 * k + 16 * jlo:
                                  2048 * g + 128 * k + 16 * jhi]
                    nc.tensor.matmul(sps[:, off:off + width], lhsT=lhsT,
                                     rhs=rhs, start=True, stop=True)
                ptil = sbp.tile([128, 1024], bf16, tag="ptil", bufs=14, name=f"pt{k}g{gi}")
                for (a, b_) in runs:
                    if noexp:
                        nc.vector.tensor_copy(ptil[:, a:b_], sps[:, a:b_])
                    else:
                        nc.scalar.activation(ptil[:, a:b_], sps[:, a:b_],
                                             mybir.ActivationFunctionType.Exp)
                if k not in state:
                    state[k] = ([], [], None)
                state[k][0].append(ptil)
                state[k][1].append(pack)

            def emit_scores(k):
                for gi in range(len(GROUPS)):
                    emit_scores_group(k, gi)

            def emit_av_group(k, gi):
                ptils, packs, av = state[k][:3]
                if av is None:
                    av = psU.tile([128, 512], f32, tag="u", name=f"av{k}")
                    state[k] = (ptils, packs, av)
                heads = GROUPS[gi]
                if True:
                    for hh, h in enumerate(heads):
                        blocks = [b for b in packs[gi] if b[0] == hh]
                        mms = []
                        for (bh, cg, jlo, jhi, off, width) in blocks:
                            if jlo == 0:
                                mms.append(((0, 16 * jhi), off, None, cg))
                            else:
                                rl = 16 * jlo
                                if rl < 64:
                                    mms.append(((rl, 64), off, (0, 32), cg))
                                    mms.append(((64, 128), off + 64 - rl,
                                                (0, 64), cg))
                                else:
                                    mms.append(((rl, 128), off, (0, rl), cg))
                        last = len(mms) - 1
                        for i, ((a, b_), off, tpos, cg) in enumerate(mms):
                            kwargs = {}
                            if tpos is not None:
                                kwargs["tile_position"] = tpos
                            nc.tensor.matmul(
                                av[a:b_, 64 * h:64 * h + 33],
                                lhsT=ptils[gi][:, off:off + (b_ - a)],
                                rhs=vsb[:, 264 * cg + 33 * h:264 * cg + 33 * (h + 1)],
                                start=(i == 0), stop=(i == last),
                                skip_group_check=True,
                                **kwargs)
            def emit_tail(k):
                ptils, packs, av = state.pop(k)
                # normalize
                av3 = av.rearrange("p (h e) -> p h e", e=64)
                zrec = small.tile([128, NUM_HEADS], f32, tag="zrec")
                nc.vector.reciprocal(zrec, av3[:, :, 32])
                zr = zrec[:, :]
                zb = bass.AP(tensor=zr.tensor, offset=zr.offset,
                             ap=[list(zr.ap[0]), [1, NUM_HEADS], [0, 32]])
                ao = small.tile([128, DIM], bf16, tag="ao")
                nc.vector.tensor_mul(
                    ao.rearrange("p (h e) -> p h e", e=32), av3[:, :, 0:32], zb)

                if stage < 4:
                    obuf = obufs[k // 4]
                    nc.vector.tensor_copy(
                        obuf[:, DIM * (k % 4):DIM * (k % 4 + 1)], ao)
                    if k % 4 == 3:
                        k0 = k - 3
                        nc.sync.dma_start(
                            out=out_ext[128 * k0:128 * (k0 + 4), :],
                            in_=obuf.rearrange("p (k d) -> p k d", k=4))
                    return
                # per-tile output chain (transpose-out and proj-out share
                # one PSUM bank: bf16 cols 0:256, f32 via bitcast after)
                combo = psU.tile([128, 1024], bf16, tag="u", name=f"combo{k}")
                tp = combo[:, 0:256]
                for fc in range(2):
                    nc.tensor.transpose(
                        tp[:, 128 * fc:128 * (fc + 1)],
                        ao[:, 128 * fc:128 * (fc + 1)], ident_sb)
                aoT = small.tile([128, DIM], bf16, tag="aoT")
                nc.vector.tensor_copy(aoT, tp)
                pr = combo[:, 512:1024].bitcast(f32)
                for fc in range(2):
                    nc.tensor.matmul(pr, lhsT=aoT[:, 128 * fc:128 * (fc + 1)],
                                     rhs=wproj_sb[fc][:, :],
                                     start=(fc == 0), stop=(fc == 1))
                obuf = obufs[k // 4]
                nc.vector.tensor_add(obuf[:, DIM * (k % 4):DIM * (k % 4 + 1)],
                                     pr, bias_rep)
                if k >= 12:
                    # last group: store per tile so the drain doesn't wait on
                    # tile 15's whole chain
                    nc.sync.dma_start(
                        out=out_ext[128 * k:128 * (k + 1), :],
                        in_=obuf[:, DIM * (k % 4):DIM * (k % 4 + 1)])
                elif k % 4 == 3:
                    # store 4 tiles in permuted (k, j, t) row order; host
                    # un-permutes to token order.
                    k0 = k - 3
                    nc.sync.dma_start(
                        out=out_ext[128 * k0:128 * (k0 + 4), :],
                        in_=obuf.rearrange("p (k d) -> p k d", k=4))

            if stage >= 2:
                # warmup: tiles 0-2 emit pair-{0,1} groups first so their
                # pair-{2,3} groups wait out the second mask/repl chain
                # behind useful work
                WARM = 3
                for k in range(WARM):
                    emit_scores_group(k, 0)
                    emit_scores_group(k, 1)
                for k in range(WARM):
                    if 2 <= k <= 14:
                        emit_v(k + 1)
                    emit_scores_group(k, 2)
                    emit_scores_group(k, 3)
                if stage >= 3:
                    for k in range(WARM - 1):
                        for gi in range(len(GROUPS)):
                            emit_av_group(k, gi)
                        emit_tail(k)
                for k in range(WARM, ntiles):
                    if 2 <= k <= 14:
                        emit_v(k + 1)
                    for gi, heads in enumerate(GROUPS):
                        emit_scores_group(k, gi)
                        if stage >= 3:
                            emit_av_group(k - 1, gi)
                    if stage >= 3:
                        emit_tail(k - 1)
                if stage >= 3:
                    for gi in range(len(GROUPS)):
                        emit_av_group(ntiles - 1, gi)
                    emit_tail(ntiles - 1)
            if stage < 3:
                for i in range(4):
                    nc.vector.memset(obufs[i], 0.0)
                    nc.sync.dma_start(
                        out=out_ext[512 * i:512 * (i + 1), :],
                        in_=obufs[i].rearrange("p (k d) -> p k d", k=4))
    nc.finalize()
    return nc


def kernel(x, w, Wqkv, Wproj, bproj, **kw):
    global LAST_EXEC_NS, LAST_RES
    assert int(w) == W
    x = np.asarray(x, dtype=np.float32)
    Wqkv = np.asarray(Wqkv, dtype=np.float32)
    Wproj = np.asarray(Wproj, dtype=np.float32)
    bproj = np.asarray(bproj, dtype=np.float32)

    if "prog" not in _CACHE:
        _CACHE["prog"] = _build_program()
        _CACHE["consts"] = _host_consts()
    nc = _CACHE["prog"]
    maskr, oh, ident = _CACHE["consts"]
    wqkvr = _reorder_weights(Wqkv)

    in_maps = []
    wqkvr = wqkvr.astype(bf)
    Wproj_bf = Wproj.astype(bf)
    for b in range(B):
        in_maps.append({
            "x": np.ascontiguousarray(x[b]),
            "wqkvr": wqkvr,
            "wproj": Wproj_bf,
            "bproj": bproj,
            "maskr": maskr,
            "oh": oh,
            "ident": ident,
        })
    res = bass_utils.run_bass_kernel_spmd(nc, in_maps, list(range(B)))
    LAST_RES = res
    LAST_EXEC_NS = res.exec_time_ns
    outs = []
    for b in range(B):
        raw = np.asarray(res.results[b]["out"]).astype(np.float32)
        out = np.empty((N, DIM), dtype=np.float32)
        # tiles 0-11: groups of 4; row within group = (j*16+t)*4 + kk
        g3 = raw[:1536].reshape(3, 8, 16, 4, DIM)      # (g, j, t, kk, d)
        # token n = 128t + 8*(4g+kk) + j
        out.reshape(16, 16, 8, DIM)[:, :12] = np.transpose(
            g3, (2, 0, 3, 1, 4)).reshape(16, 12, 8, DIM)
        # tiles 12-15: per-tile rows (j*16+t)
        last = raw[1536:].reshape(4, 8, 16, DIM)        # (kk, j, t, d)
        out.reshape(16, 16, 8, DIM)[:, 12:] = np.transpose(
            last, (2, 0, 1, 3))
        outs.append(out)
    return np.stack(outs, axis=0).astype(np.float32)
